# revision 8
# baseline (speedup 1.0000x reference)
"""GNN NodeBlock (segment_sum scatter + 2-layer MLP) on 8 Trainium2 cores.

v2 strategy (edge/vertex partitioning by receiver range, fp8 payload):
 - 2 graphs x 4 cores each; core owns a 12500-node range and all edges
   whose receiver falls in that range.
 - Nodes grouped in 128-node blocks (98/core), each split into 4 stripes
   of 32 nodes. Host buckets edges by stripe; each stripe owns 3 fixed
   128-edge chunks (12 chunks/block). Stripes overflowing 384 edges are
   pre-compressed host-side (tail summed by receiver, <=32 rows).
 - Edge features are quantized to fp8 e4m3 with host-side error
   feedback: the fp8 rounding residual of each edge is carried into the
   next edge targeting the same (node, feature), so the device's exact
   fp32 PSUM sum of the quantized edges matches the fp32 sum to ~1 ulp
   of a single fp8 value. Halves DMA bytes vs bf16 at ~6e-3 rel error.
 - Edge ids are stored as fp8 BIT PATTERNS 1..32 (distinct values, not
   integers) and compared against a host-provided fp8 pattern iota via
   DVE is_equal to build the per-chunk one-hot [128 edge, 32 node].
 - Device, per chunk: scatter via matmul into agg_T [De, 128]
   (feature-major, fp8 lhsT/rhs, fp32 PSUM), then the MLP feature-major
   in bf16, batched over groups of 4 blocks.
 - Engine split: one-hot + agg PSUM->SBUF copy on Vector, relu+bias and
   out bias on Scalar, payload DMA on Sync queue, node/out DMA on
   GpSimd queue.
"""
import numpy as np
import ml_dtypes

import concourse.bacc as bacc
import concourse.mybir as mybir
from concourse.tile import TileContext
from concourse.bass_utils import run_bass_kernel_spmd

B, N, E = 2, 50000, 512000
De, Dv, H, Do = 128, 128, 256, 128
NCORES = 8
CPG = 4                    # cores per graph
NPC = N // CPG             # 12500 nodes per core
NB = (NPC + 127) // 128    # 98 blocks per core
SW = 32                    # stripe width: nodes per one-hot stripe
NSPB = 4                   # stripes per block
CPS = 3                    # 128-edge chunks per stripe
NSC = NSPB * CPS           # 12 chunks per block
SCAP = CPS * 128           # 384 edge slots per stripe
FOLD_KEEP = SCAP - SW      # fold tail beyond 352 edges (fold adds <=32 rows)
IDOFF = NSC * 128          # ids start at column 1536
PAYW = IDOFF + NSC + 4     # 1552 bytes per partition row (4 pad)
GRP = 4                    # blocks per MLP group
GROUPS = [GRP] * (NB // GRP) + ([NB % GRP] if NB % GRP else [])

F32 = mybir.dt.float32
BF16 = mybir.dt.bfloat16
FP8 = mybir.dt.float8e4
NP_FP8 = ml_dtypes.float8_e4m3fn
NP_BF16 = ml_dtypes.bfloat16


def _build_nc():
    nc = bacc.Bacc("TRN2", target_bir_lowering=False)
    payload = nc.dram_tensor("payload", [len(GROUPS), 128, GRP * PAYW], FP8, kind="ExternalInput")
    nodes_g = nc.dram_tensor("nodes_g", [len(GROUPS), 128, GRP * 128], BF16, kind="ExternalInput")
    w1 = nc.dram_tensor("w1", [128, 512], BF16, kind="ExternalInput")   # [p, dk*256+hm*128+j] = W1[dk*128+p, hm*128+j]
    w2 = nc.dram_tensor("w2", [128, 256], BF16, kind="ExternalInput")   # [p, hm*128+j] = W2[hm*128+p, j]
    b1 = nc.dram_tensor("b1", [128, 2], F32, kind="ExternalInput")      # [p, hm] = b1[hm*128+p]
    b2 = nc.dram_tensor("b2", [128, 1], F32, kind="ExternalInput")
    iota8 = nc.dram_tensor("iota8", [128, SW], FP8, kind="ExternalInput")
    out_g = nc.dram_tensor("out_g", [len(GROUPS), 128, GRP * 128], BF16, kind="ExternalOutput")

    with TileContext(nc) as tc:
        with tc.tile_pool(name="const", bufs=1) as cp, \
             tc.tile_pool(name="pay", bufs=12) as payp, \
             tc.tile_pool(name="oh", bufs=4) as ohp, \
             tc.tile_pool(name="nod", bufs=8) as nodp, \
             tc.tile_pool(name="agg4", bufs=3) as aggp, \
             tc.tile_pool(name="hsb", bufs=3) as hp, \
             tc.tile_pool(name="osb", bufs=4) as op_, \
             tc.tile_pool(name="psA", bufs=2, space="PSUM") as psA, \
             tc.tile_pool(name="psH", bufs=2, space="PSUM") as psH, \
             tc.tile_pool(name="psO", bufs=2, space="PSUM") as psO:
            # iota first (first one-hot depends on it), then weights on the
            # gpsimd queue so the sync queue starts streaming payload groups
            # immediately.
            iota_sb = cp.tile([128, SW], FP8)
            nc.sync.dma_start(out=iota_sb[:], in_=iota8[:, :])
            w1_sb = cp.tile([128, 512], BF16)
            nc.gpsimd.dma_start(out=w1_sb[:], in_=w1[:, :])
            w2_sb = cp.tile([128, 256], BF16)
            nc.gpsimd.dma_start(out=w2_sb[:], in_=w2[:, :])
            b1_sb = cp.tile([128, 2], F32)
            nc.gpsimd.dma_start(out=b1_sb[:], in_=b1[:, :])
            b2_sb = cp.tile([128, 1], F32)
            nc.gpsimd.dma_start(out=b2_sb[:], in_=b2[:, :])

            for gi, g_sz in enumerate(GROUPS):
                nod = nodp.tile([128, GRP * 128], BF16)
                nc.gpsimd.dma_start(out=nod[:, :g_sz * 128],
                                    in_=nodes_g[gi, :, :g_sz * 128])
                pay_g = payp.tile([128, GRP * PAYW], FP8)
                nc.sync.dma_start(out=pay_g[:, :g_sz * PAYW],
                                  in_=payload[gi, :, :g_sz * PAYW])

                # one-hot for the whole group in one DVE op:
                # oh_g[p, g, c, n] = (ids[p, g, c] == iota[p, n])
                oh_g = ohp.tile([128, GRP * NSC * SW], FP8)
                ids_ap = (pay_g[:, :g_sz * PAYW]
                          .rearrange("p (g w) -> p g w", g=g_sz)
                          [:, :, IDOFF:IDOFF + NSC]
                          .to_broadcast([128, g_sz, NSC, SW]))
                iota_ap = (iota_sb[:, None, :SW]
                           .to_broadcast([128, g_sz * NSC, SW])
                           .rearrange("p (g c) n -> p g c n", g=g_sz))
                nc.vector.tensor_tensor(
                    out=oh_g[:, :g_sz * NSC * SW].rearrange(
                        "p (g c n) -> p g c n", g=g_sz, c=NSC),
                    in0=ids_ap,
                    in1=iota_ap,
                    op=mybir.AluOpType.is_equal,
                )

                agg_ps = psA.tile([128, GRP * 128], F32, space="PSUM")
                for g in range(g_sz):
                    po = g * PAYW
                    oo = g * NSC * SW
                    for s in range(NSPB):
                        col = g * 128 + s * SW
                        for k in range(CPS):
                            pc = s * CPS + k
                            nc.tensor.matmul(
                                out=agg_ps[:, col:col + SW],
                                lhsT=pay_g[:, po + pc * 128:po + (pc + 1) * 128],
                                rhs=oh_g[:, oo + pc * SW:oo + (pc + 1) * SW],
                                start=(k == 0),
                                stop=(k == CPS - 1),
                            )

                agg_sb = aggp.tile([128, GRP * 128], BF16)
                if gi % 2 == 0:
                    nc.vector.tensor_copy(agg_sb[:, :g_sz * 128], agg_ps[:, :g_sz * 128])
                else:
                    nc.scalar.copy(agg_sb[:, :g_sz * 128], agg_ps[:, :g_sz * 128])

                hps = psH.tile([128, 2 * GRP * 128], F32, space="PSUM")
                for hm in range(2):
                    for dk in range(2):
                        nc.tensor.matmul(
                            out=hps[:, hm * GRP * 128:hm * GRP * 128 + g_sz * 128],
                            lhsT=w1_sb[:, dk * 256 + hm * 128:dk * 256 + (hm + 1) * 128],
                            rhs=(agg_sb[:, :g_sz * 128] if dk == 0 else nod[:, :g_sz * 128]),
                            start=(dk == 0),
                            stop=(dk == 1),
                        )
                h_sb = hp.tile([128, 2 * GRP * 128], BF16)
                for hm in range(2):
                    nc.scalar.activation(
                        out=h_sb[:, hm * GRP * 128:hm * GRP * 128 + g_sz * 128],
                        in_=hps[:, hm * GRP * 128:hm * GRP * 128 + g_sz * 128],
                        func=mybir.ActivationFunctionType.Relu,
                        bias=b1_sb[:, hm:hm + 1],
                    )

                ops = psO.tile([128, GRP * 128], F32, space="PSUM")
                for hm in range(2):
                    nc.tensor.matmul(
                        out=ops[:, :g_sz * 128],
                        lhsT=w2_sb[:, hm * 128:(hm + 1) * 128],
                        rhs=h_sb[:, hm * GRP * 128:hm * GRP * 128 + g_sz * 128],
                        start=(hm == 0),
                        stop=(hm == 1),
                    )
                o_sb = op_.tile([128, GRP * 128], BF16)
                if gi % 2 == 0:
                    nc.scalar.activation(
                        out=o_sb[:, :g_sz * 128],
                        in_=ops[:, :g_sz * 128],
                        func=mybir.ActivationFunctionType.Identity,
                        bias=b2_sb[:, 0:1],
                    )
                else:
                    nc.vector.tensor_scalar(
                        out=o_sb[:, :g_sz * 128],
                        in0=ops[:, :g_sz * 128],
                        scalar1=b2_sb[:, 0:1],
                        scalar2=None,
                        op0=mybir.AluOpType.add,
                    )
                nc.sync.dma_start(out=out_g[gi, :, :g_sz * 128],
                                  in_=o_sb[:, :g_sz * 128])
    nc.compile()
    return nc


def _quantize_feedback(efeat, local):
    """fp8-quantize edge rows with per-(node,feature) error feedback.

    Rows sharing a receiver node are quantized sequentially, carrying the
    rounding residual into the next row, so the fp32 sum of the quantized
    rows tracks the fp32 sum of the originals to ~one fp8 rounding error.
    """
    order = np.argsort(local, kind="stable")
    ls = local[order]
    counts = np.bincount(ls, minlength=NPC)
    offs = np.zeros(NPC, np.int64)
    np.cumsum(counts[:-1], out=offs[1:])
    rank = np.arange(len(ls)) - offs[ls]
    q = np.empty((len(ls), De), NP_FP8)
    carry = np.zeros((NPC, De), np.float32)
    es = efeat[order]
    maxr = int(rank.max()) + 1 if len(rank) else 0
    for r in range(maxr):
        m = rank == r
        idx = ls[m]
        v = es[m] + carry[idx]
        qq = v.astype(NP_FP8)
        carry[idx] = v - qq.astype(np.float32)
        q[m] = qq
    out = np.empty_like(q)
    out[order] = q
    return out


def _prep_core(efeat, local, nodes_g_core):
    """Build one core's payload from its edges (efeat fp32, local in [0,NPC))."""
    blk = local >> 7
    w128 = local & 127
    stripe = blk * NSPB + (w128 >> 5)   # global stripe id, [0, NB*4)
    w32 = w128 & 31
    nstripes = NB * NSPB
    counts = np.bincount(stripe, minlength=nstripes)
    if (counts > SCAP).any():
        # tail-fold overflowing stripes: sum the last (c-FOLD_KEEP) edges
        # by receiver (<=32 rows), keeping total <= FOLD_KEEP + 32 = SCAP
        keep = np.ones(len(stripe), bool)
        extra_f, extra_s, extra_l = [], [], []
        for ob in np.nonzero(counts > SCAP)[0]:
            idxs = np.nonzero(stripe == ob)[0]
            tail = idxs[FOLD_KEEP:]
            keep[tail] = False
            seg = np.zeros((SW, De), np.float32)
            np.add.at(seg, w32[tail], efeat[tail])
            rows = np.unique(w32[tail])
            extra_f.append(seg[rows])
            extra_s.append(np.full(len(rows), ob, stripe.dtype))
            extra_l.append(rows.astype(w32.dtype))
        efeat = np.concatenate([efeat[keep]] + extra_f)
        stripe = np.concatenate([stripe[keep]] + extra_s)
        w32 = np.concatenate([w32[keep]] + extra_l)
        counts = np.bincount(stripe, minlength=nstripes)

    qfeat = _quantize_feedback(efeat, stripe // NSPB * 128 + (stripe % NSPB) * SW + w32)

    order = np.argsort(stripe, kind="stable")
    str_s = stripe[order]
    offs = np.zeros(nstripes, np.int64)
    np.cumsum(counts[:-1], out=offs[1:])
    pos = np.arange(len(str_s)) - offs[str_s]
    blk_s = str_s // NSPB
    s_s = str_s % NSPB
    pc = s_s * CPS + pos // 128        # physical chunk 0..11
    prow = pos % 128

    payload = np.zeros((NB, 128, PAYW), np.uint8)
    feat_view = payload[:, :, :IDOFF].reshape(NB, 128, NSC, 128).view(NP_FP8)
    feat_view[blk_s, prow, pc, :] = qfeat[order]
    id_view = payload[:, :, IDOFF:IDOFF + NSC]
    id_view[blk_s, prow, pc] = (w32[order] + 1).astype(np.uint8)  # fp8 patterns
    ng = len(GROUPS)
    pay_pad = np.zeros((ng * GRP, 128, PAYW), np.uint8)
    pay_pad[:NB] = payload
    payload_gm = np.ascontiguousarray(
        pay_pad.reshape(ng, GRP, 128, PAYW).transpose(0, 2, 1, 3)
    ).reshape(ng, 128, GRP * PAYW).view(NP_FP8)
    return {"payload": payload_gm, "nodes_g": nodes_g_core}


def kernel(edge_data, node_data, W1, b1, W2, b2, receiver_ids, _trace=False):
    edge_data = np.asarray(edge_data, np.float32)
    node_data = np.asarray(node_data, np.float32)
    W1 = np.asarray(W1, np.float32)
    b1 = np.asarray(b1, np.float32)
    W2 = np.asarray(W2, np.float32)
    b2 = np.asarray(b2, np.float32)
    rid = np.asarray(receiver_ids).astype(np.int64)

    w1_dev = np.ascontiguousarray(
        W1.reshape(2, 128, H).transpose(1, 0, 2).reshape(128, 2 * H)).astype(NP_BF16)
    w2_dev = np.ascontiguousarray(
        W2.reshape(2, 128, Do).transpose(1, 0, 2).reshape(128, 2 * Do)).astype(NP_BF16)
    b1_dev = np.ascontiguousarray(b1.reshape(2, 128).T)
    b2_dev = np.ascontiguousarray(b2.reshape(128, 1))
    iota_dev = np.ascontiguousarray(np.broadcast_to(
        np.arange(1, SW + 1, dtype=np.uint8), (128, SW))).view(NP_FP8)

    ng = len(GROUPS)
    in_maps = []
    for core in range(NCORES):
        g, part = divmod(core, CPG)
        base = part * NPC
        sel = (rid[g] >= base) & (rid[g] < base + NPC)
        local = rid[g][sel] - base
        efeat = edge_data[g][sel]

        nd = np.zeros((ng * GRP * 128, Dv), np.float32)
        nd[:NPC] = node_data[g, base:base + NPC]
        # [ng, 128 d, GRP*128 n]: group-contiguous, feature-major
        nodes_g_core = np.ascontiguousarray(
            nd.reshape(ng, GRP * 128, Dv).transpose(0, 2, 1)).astype(NP_BF16)

        m = _prep_core(efeat, local, nodes_g_core)
        m.update({"w1": w1_dev, "w2": w2_dev, "b1": b1_dev, "b2": b2_dev,
                  "iota8": iota_dev})
        in_maps.append(m)

    nc = _build_nc()
    res = run_bass_kernel_spmd(nc, in_maps, core_ids=list(range(NCORES)),
                               trace=_trace)

    out = np.empty((B, N, Do), np.float32)
    for core in range(NCORES):
        g, part = divmod(core, CPG)
        og = res.results[core]["out_g"].astype(np.float32)  # [ng, 128 o, GRP*128 j]
        on = og.transpose(0, 2, 1).reshape(ng * GRP * 128, Do)
        out[g, part * NPC:(part + 1) * NPC] = on[:NPC]
    if _trace:
        kernel._last = res
    return out


# revision 9
# speedup vs baseline: 1.0605x; 1.0605x over previous
"""GNN NodeBlock (segment_sum scatter + 2-layer MLP) on 8 Trainium2 cores.

v2 strategy (edge/vertex partitioning by receiver range, fp8 payload):
 - 2 graphs x 4 cores each; core owns a 12500-node range and all edges
   whose receiver falls in that range.
 - Nodes grouped in 128-node blocks (98/core), each split into 4 stripes
   of 32 nodes. Host buckets edges by stripe; each stripe owns 3 fixed
   128-edge chunks (12 chunks/block). Stripes overflowing 384 edges are
   pre-compressed host-side (tail summed by receiver, <=32 rows).
 - Edge features are quantized to fp8 e4m3 with host-side error
   feedback: the fp8 rounding residual of each edge is carried into the
   next edge targeting the same (node, feature), so the device's exact
   fp32 PSUM sum of the quantized edges matches the fp32 sum to ~1 ulp
   of a single fp8 value. Halves DMA bytes vs bf16 at ~6e-3 rel error.
 - Edge ids are stored as fp8 BIT PATTERNS 1..32 (distinct values, not
   integers) and compared against a host-provided fp8 pattern iota via
   DVE is_equal to build the per-chunk one-hot [128 edge, 32 node].
 - Device, per chunk: scatter via matmul into agg_T [De, 128]
   (feature-major, fp8 lhsT/rhs, fp32 PSUM), then the MLP feature-major
   in bf16, batched over groups of 4 blocks.
 - Engine split: one-hot + agg PSUM->SBUF copy on Vector, relu+bias and
   out bias on Scalar, payload DMA on Sync queue, node/out DMA on
   GpSimd queue.
"""
import numpy as np
import ml_dtypes

import concourse.bacc as bacc
import concourse.mybir as mybir
from concourse.tile import TileContext
from concourse.bass_utils import run_bass_kernel_spmd

B, N, E = 2, 50000, 512000
De, Dv, H, Do = 128, 128, 256, 128
NCORES = 8
CPG = 4                    # cores per graph
NPC = N // CPG             # 12500 nodes per core
NB = (NPC + 127) // 128    # 98 blocks per core
SW = 32                    # stripe width: nodes per one-hot stripe
NSPB = 4                   # stripes per block
CPS = 3                    # 128-edge chunks per stripe
NSC = NSPB * CPS           # 12 chunks per block
SCAP = CPS * 128           # 384 edge slots per stripe
FOLD_KEEP = SCAP - SW      # fold tail beyond 352 edges (fold adds <=32 rows)
IDOFF = NSC * 128          # ids start at column 1536
PAYW = IDOFF + NSC + 4     # 1552 bytes per partition row (4 pad)
GRP = 4                    # blocks per MLP group
GROUPS = [GRP] * (NB // GRP) + ([NB % GRP] if NB % GRP else [])

F32 = mybir.dt.float32
BF16 = mybir.dt.bfloat16
FP8 = mybir.dt.float8e4
NP_FP8 = ml_dtypes.float8_e4m3fn
NP_BF16 = ml_dtypes.bfloat16


def _build_nc():
    nc = bacc.Bacc("TRN2", target_bir_lowering=False)
    payload = nc.dram_tensor("payload", [len(GROUPS), 128, GRP * PAYW], FP8, kind="ExternalInput")
    nodes_g = nc.dram_tensor("nodes_g", [len(GROUPS), 128, GRP * 128], BF16, kind="ExternalInput")
    w1 = nc.dram_tensor("w1", [128, 512], BF16, kind="ExternalInput")   # [p, dk*256+hm*128+j] = W1[dk*128+p, hm*128+j]
    w2 = nc.dram_tensor("w2", [128, 256], BF16, kind="ExternalInput")   # [p, hm*128+j] = W2[hm*128+p, j]
    b1 = nc.dram_tensor("b1", [128, 2], F32, kind="ExternalInput")      # [p, hm] = b1[hm*128+p]
    b2 = nc.dram_tensor("b2", [128, 1], F32, kind="ExternalInput")
    iota8 = nc.dram_tensor("iota8", [128, SW], FP8, kind="ExternalInput")
    out_g = nc.dram_tensor("out_g", [len(GROUPS), 128, GRP * 128], BF16, kind="ExternalOutput")

    with TileContext(nc) as tc:
        with tc.tile_pool(name="const", bufs=1) as cp, \
             tc.tile_pool(name="pay", bufs=12) as payp, \
             tc.tile_pool(name="oh", bufs=4) as ohp, \
             tc.tile_pool(name="nod", bufs=8) as nodp, \
             tc.tile_pool(name="agg4", bufs=3) as aggp, \
             tc.tile_pool(name="hsb", bufs=3) as hp, \
             tc.tile_pool(name="osb", bufs=4) as op_, \
             tc.tile_pool(name="psA", bufs=2, space="PSUM") as psA, \
             tc.tile_pool(name="psH", bufs=2, space="PSUM") as psH, \
             tc.tile_pool(name="psO", bufs=2, space="PSUM") as psO:
            # iota first (first one-hot depends on it), then weights on the
            # gpsimd queue so the sync queue starts streaming payload groups
            # immediately.
            iota_sb = cp.tile([128, SW], FP8)
            nc.sync.dma_start(out=iota_sb[:], in_=iota8[:, :])
            w1_sb = cp.tile([128, 512], BF16)
            nc.gpsimd.dma_start(out=w1_sb[:], in_=w1[:, :])
            w2_sb = cp.tile([128, 256], BF16)
            nc.gpsimd.dma_start(out=w2_sb[:], in_=w2[:, :])
            b1_sb = cp.tile([128, 2], F32)
            nc.gpsimd.dma_start(out=b1_sb[:], in_=b1[:, :])
            b2_sb = cp.tile([128, 1], F32)
            nc.gpsimd.dma_start(out=b2_sb[:], in_=b2[:, :])

            for gi, g_sz in enumerate(GROUPS):
                nod = nodp.tile([128, GRP * 128], BF16)
                nc.gpsimd.dma_start(out=nod[:, :g_sz * 128],
                                    in_=nodes_g[gi, :, :g_sz * 128])
                pay_g = payp.tile([128, GRP * PAYW], FP8)
                nc.sync.dma_start(out=pay_g[:, :g_sz * PAYW],
                                  in_=payload[gi, :, :g_sz * PAYW])

                # one-hot for the whole group in one DVE op:
                # oh_g[p, g, c, n] = (ids[p, g, c] == iota[p, n])
                oh_g = ohp.tile([128, GRP * NSC * SW], FP8)
                ids_ap = (pay_g[:, :g_sz * PAYW]
                          .rearrange("p (g w) -> p g w", g=g_sz)
                          [:, :, IDOFF:IDOFF + NSC]
                          .to_broadcast([128, g_sz, NSC, SW]))
                iota_ap = (iota_sb[:, None, :SW]
                           .to_broadcast([128, g_sz * NSC, SW])
                           .rearrange("p (g c) n -> p g c n", g=g_sz))
                nc.vector.tensor_tensor(
                    out=oh_g[:, :g_sz * NSC * SW].rearrange(
                        "p (g c n) -> p g c n", g=g_sz, c=NSC),
                    in0=ids_ap,
                    in1=iota_ap,
                    op=mybir.AluOpType.is_equal,
                )

                agg_ps = psA.tile([128, GRP * 128], F32, space="PSUM")
                for g in range(g_sz):
                    po = g * PAYW
                    oo = g * NSC * SW
                    for s in range(NSPB):
                        col = g * 128 + s * SW
                        for k in range(CPS):
                            pc = s * CPS + k
                            nc.tensor.matmul(
                                out=agg_ps[:, col:col + SW],
                                lhsT=pay_g[:, po + pc * 128:po + (pc + 1) * 128],
                                rhs=oh_g[:, oo + pc * SW:oo + (pc + 1) * SW],
                                start=(k == 0),
                                stop=(k == CPS - 1),
                            )

                agg_sb = aggp.tile([128, GRP * 128], BF16)
                if gi % 2 == 0:
                    nc.vector.tensor_copy(agg_sb[:, :g_sz * 128], agg_ps[:, :g_sz * 128])
                else:
                    nc.scalar.copy(agg_sb[:, :g_sz * 128], agg_ps[:, :g_sz * 128])

                hps = psH.tile([128, 2 * GRP * 128], F32, space="PSUM")
                for hm in range(2):
                    for dk in range(2):
                        nc.tensor.matmul(
                            out=hps[:, hm * GRP * 128:hm * GRP * 128 + g_sz * 128],
                            lhsT=w1_sb[:, dk * 256 + hm * 128:dk * 256 + (hm + 1) * 128],
                            rhs=(agg_sb[:, :g_sz * 128] if dk == 0 else nod[:, :g_sz * 128]),
                            start=(dk == 0),
                            stop=(dk == 1),
                        )
                h_sb = hp.tile([128, 2 * GRP * 128], BF16)
                for hm in range(2):
                    nc.scalar.activation(
                        out=h_sb[:, hm * GRP * 128:hm * GRP * 128 + g_sz * 128],
                        in_=hps[:, hm * GRP * 128:hm * GRP * 128 + g_sz * 128],
                        func=mybir.ActivationFunctionType.Relu,
                        bias=b1_sb[:, hm:hm + 1],
                    )

                ops = psO.tile([128, GRP * 128], F32, space="PSUM")
                for hm in range(2):
                    nc.tensor.matmul(
                        out=ops[:, :g_sz * 128],
                        lhsT=w2_sb[:, hm * 128:(hm + 1) * 128],
                        rhs=h_sb[:, hm * GRP * 128:hm * GRP * 128 + g_sz * 128],
                        start=(hm == 0),
                        stop=(hm == 1),
                    )
                o_sb = op_.tile([128, GRP * 128], BF16)
                if gi % 2 == 0:
                    nc.scalar.activation(
                        out=o_sb[:, :g_sz * 128],
                        in_=ops[:, :g_sz * 128],
                        func=mybir.ActivationFunctionType.Identity,
                        bias=b2_sb[:, 0:1],
                    )
                else:
                    nc.vector.tensor_scalar(
                        out=o_sb[:, :g_sz * 128],
                        in0=ops[:, :g_sz * 128],
                        scalar1=b2_sb[:, 0:1],
                        scalar2=None,
                        op0=mybir.AluOpType.add,
                    )
                nc.gpsimd.dma_start(out=out_g[gi, :, :g_sz * 128],
                                    in_=o_sb[:, :g_sz * 128])
    nc.compile()
    return nc


def _quantize_feedback(efeat, local):
    """fp8-quantize edge rows with per-(node,feature) error feedback.

    Rows sharing a receiver node are quantized sequentially, carrying the
    rounding residual into the next row, so the fp32 sum of the quantized
    rows tracks the fp32 sum of the originals to ~one fp8 rounding error.
    """
    order = np.argsort(local, kind="stable")
    ls = local[order]
    counts = np.bincount(ls, minlength=NPC)
    offs = np.zeros(NPC, np.int64)
    np.cumsum(counts[:-1], out=offs[1:])
    rank = np.arange(len(ls)) - offs[ls]
    q = np.empty((len(ls), De), NP_FP8)
    carry = np.zeros((NPC, De), np.float32)
    es = efeat[order]
    maxr = int(rank.max()) + 1 if len(rank) else 0
    for r in range(maxr):
        m = rank == r
        idx = ls[m]
        v = es[m] + carry[idx]
        qq = v.astype(NP_FP8)
        carry[idx] = v - qq.astype(np.float32)
        q[m] = qq
    out = np.empty_like(q)
    out[order] = q
    return out


def _prep_core(efeat, local, nodes_g_core):
    """Build one core's payload from its edges (efeat fp32, local in [0,NPC))."""
    blk = local >> 7
    w128 = local & 127
    stripe = blk * NSPB + (w128 >> 5)   # global stripe id, [0, NB*4)
    w32 = w128 & 31
    nstripes = NB * NSPB
    counts = np.bincount(stripe, minlength=nstripes)
    if (counts > SCAP).any():
        # tail-fold overflowing stripes: sum the last (c-FOLD_KEEP) edges
        # by receiver (<=32 rows), keeping total <= FOLD_KEEP + 32 = SCAP
        keep = np.ones(len(stripe), bool)
        extra_f, extra_s, extra_l = [], [], []
        for ob in np.nonzero(counts > SCAP)[0]:
            idxs = np.nonzero(stripe == ob)[0]
            tail = idxs[FOLD_KEEP:]
            keep[tail] = False
            seg = np.zeros((SW, De), np.float32)
            np.add.at(seg, w32[tail], efeat[tail])
            rows = np.unique(w32[tail])
            extra_f.append(seg[rows])
            extra_s.append(np.full(len(rows), ob, stripe.dtype))
            extra_l.append(rows.astype(w32.dtype))
        efeat = np.concatenate([efeat[keep]] + extra_f)
        stripe = np.concatenate([stripe[keep]] + extra_s)
        w32 = np.concatenate([w32[keep]] + extra_l)
        counts = np.bincount(stripe, minlength=nstripes)

    qfeat = _quantize_feedback(efeat, stripe // NSPB * 128 + (stripe % NSPB) * SW + w32)

    order = np.argsort(stripe, kind="stable")
    str_s = stripe[order]
    offs = np.zeros(nstripes, np.int64)
    np.cumsum(counts[:-1], out=offs[1:])
    pos = np.arange(len(str_s)) - offs[str_s]
    blk_s = str_s // NSPB
    s_s = str_s % NSPB
    pc = s_s * CPS + pos // 128        # physical chunk 0..11
    prow = pos % 128

    payload = np.zeros((NB, 128, PAYW), np.uint8)
    feat_view = payload[:, :, :IDOFF].reshape(NB, 128, NSC, 128).view(NP_FP8)
    feat_view[blk_s, prow, pc, :] = qfeat[order]
    id_view = payload[:, :, IDOFF:IDOFF + NSC]
    id_view[blk_s, prow, pc] = (w32[order] + 1).astype(np.uint8)  # fp8 patterns
    ng = len(GROUPS)
    pay_pad = np.zeros((ng * GRP, 128, PAYW), np.uint8)
    pay_pad[:NB] = payload
    payload_gm = np.ascontiguousarray(
        pay_pad.reshape(ng, GRP, 128, PAYW).transpose(0, 2, 1, 3)
    ).reshape(ng, 128, GRP * PAYW).view(NP_FP8)
    return {"payload": payload_gm, "nodes_g": nodes_g_core}


def kernel(edge_data, node_data, W1, b1, W2, b2, receiver_ids, _trace=False):
    edge_data = np.asarray(edge_data, np.float32)
    node_data = np.asarray(node_data, np.float32)
    W1 = np.asarray(W1, np.float32)
    b1 = np.asarray(b1, np.float32)
    W2 = np.asarray(W2, np.float32)
    b2 = np.asarray(b2, np.float32)
    rid = np.asarray(receiver_ids).astype(np.int64)

    w1_dev = np.ascontiguousarray(
        W1.reshape(2, 128, H).transpose(1, 0, 2).reshape(128, 2 * H)).astype(NP_BF16)
    w2_dev = np.ascontiguousarray(
        W2.reshape(2, 128, Do).transpose(1, 0, 2).reshape(128, 2 * Do)).astype(NP_BF16)
    b1_dev = np.ascontiguousarray(b1.reshape(2, 128).T)
    b2_dev = np.ascontiguousarray(b2.reshape(128, 1))
    iota_dev = np.ascontiguousarray(np.broadcast_to(
        np.arange(1, SW + 1, dtype=np.uint8), (128, SW))).view(NP_FP8)

    ng = len(GROUPS)
    in_maps = []
    for core in range(NCORES):
        g, part = divmod(core, CPG)
        base = part * NPC
        sel = (rid[g] >= base) & (rid[g] < base + NPC)
        local = rid[g][sel] - base
        efeat = edge_data[g][sel]

        nd = np.zeros((ng * GRP * 128, Dv), np.float32)
        nd[:NPC] = node_data[g, base:base + NPC]
        # [ng, 128 d, GRP*128 n]: group-contiguous, feature-major
        nodes_g_core = np.ascontiguousarray(
            nd.reshape(ng, GRP * 128, Dv).transpose(0, 2, 1)).astype(NP_BF16)

        m = _prep_core(efeat, local, nodes_g_core)
        m.update({"w1": w1_dev, "w2": w2_dev, "b1": b1_dev, "b2": b2_dev,
                  "iota8": iota_dev})
        in_maps.append(m)

    nc = _build_nc()
    res = run_bass_kernel_spmd(nc, in_maps, core_ids=list(range(NCORES)),
                               trace=_trace)

    out = np.empty((B, N, Do), np.float32)
    for core in range(NCORES):
        g, part = divmod(core, CPG)
        og = res.results[core]["out_g"].astype(np.float32)  # [ng, 128 o, GRP*128 j]
        on = og.transpose(0, 2, 1).reshape(ng * GRP * 128, Do)
        out[g, part * NPC:(part + 1) * NPC] = on[:NPC]
    if _trace:
        kernel._last = res
    return out
